# revision 1
# baseline (speedup 1.0000x reference)
"""Trainium2 Bass kernel for nn_KernelGraphAttentionNetwork.

Strategy (8 NeuronCores):
  Sharding: batch (2 groups of 4 cores) x S1-quarters (4 query sentences
  per core).  Each core UPLOADS ONLY ITS OWN query-column shard (768x256
  bf16, ~384KB) and the full key matrix is assembled ON DEVICE with an
  AllGather over its 4-core group -- host->device traffic is ~3MB total
  instead of ~15MB of replicated uploads.

  Edge kernel on device, for the core's 256 query tokens x all 1024 key
  tokens:
    sim   = rhat_q^T @ rhat_all                    (PE, bf16, contract D=768)
    RBF:  all 10 kernels share sigma=0.1 and equally spaced mu, so
          t_k = exp(-50(s-mu_k)^2) collapses to a geometric chain:
            c_0 = exp(-50(s-0.9)^2)           (ScalarE: Square + Exp)
            w   = exp(-20s), w2 = exp(-40s)   (ScalarE: Exp)
            w3  = w*w2, c_m = c_{m-3}*w3 ...  (DVE/GpSimd bf16 muls)
          with c_m = t_{m+1} * exp(-C_m), C_m = 40.5 - 50*mu_m^2 a
          per-kernel constant folded into the clamp threshold and the
          (softmax-invariant) logit constant.
    pool  = segmented sum over T2                  (DVE/GpSimd reduces)
    Ke    = ln(max(pool, 1e-6*exp(-C)))            (DVE max + ScalarE Ln)
    logit = sum_k Ke*w_sel[k]                      (DVE mul + reduce)
  This needs 4 ScalarE activation passes per 128x1024 tile instead of the
  naive 20 (Square+Exp per kernel).

  Host: normalizes reps, builds bf16 shards, runs the tiny coupled tail
  (T1-softmax, z_hat, gating MLP, beta softmax over S1, label head, node
  kernel) in float32.

  The shard_map/jit executable is built ONCE at module import (including
  a warmup execution so walrus compile + NEFF load + comm setup are off
  the per-call path).
"""

import os
import sys

import numpy as np

KERNEL = 11
B, S, T, D = 2, 16, 64, 768
EPS = 1e-6
CLAMP_MIN = 1e-6
N_CORES = 8
NK = KERNEL - 1  # k=0 (exact-match, sigma=1e-3) is constant over T1 -> softmax-invariant


def _kernel_mus(n):
    mus = [1.0]
    if n == 1:
        return mus
    b = 2.0 / (n - 1)
    mus.append(1.0 - b / 2.0)
    for i in range(1, n - 1):
        mus.append(mus[i] - b)
    return mus


MU = np.asarray(_kernel_mus(KERNEL), dtype=np.float64)
SIGMA = np.asarray([0.001] + [0.1] * (KERNEL - 1), dtype=np.float64)

# c_m = t_{mu_m} * exp(-C_m):  c_m = c_0 * w^m with c_0 = exp(-50(s-.9)^2),
# w = exp(-20s);  completing the square gives C_m = 40.5 - 50*mu_m^2 >= 0.
_MUK = MU[1:]  # (10,) = 0.9, 0.7, ..., -0.9
_CM = 40.5 - 50.0 * _MUK**2  # (10,) >= 0, C_0 = C_9 = 0

_STATE = {}
LAST_RESULTS = None
_USE_GPSIMD = True
# Split the AllGather into two D-halves so the second half's transfer
# overlaps the first half's matmul accumulation (the Tile framework extracts
# the pipeline from data deps automatically).
_SPLIT_AG = False


# Odd chain powers are LEAVES of the even-anchor DAG; these are computed
# DIRECTLY on the (otherwise underused) ScalarE via Square(bias=-mu_m) +
# Exp, so their chain constant is 0.  Four leaves balances ScalarE
# (~13us/tile) against DVE (~14us/tile).
_DIRECT_M = (3, 5, 7, 9)
_CM_EFF = _CM.copy()
for _m in _DIRECT_M:
    _CM_EFF[_m] = 0.0


def _build_consts(w_sel):
    """(325,) f32: [0:160] w_sel broadcast per (j,k); [160:320] clamp
    thresholds; [320:] the -mu Square biases (m=0, then _DIRECT_M)."""
    wsel_pat = np.tile(np.asarray(w_sel, dtype=np.float64)[1:, 0], S)
    thr_pat = np.tile(CLAMP_MIN * np.exp(-_CM_EFF), S)
    biases = [-_MUK[0]] + [-_MUK[m] for m in _DIRECT_M]
    return np.concatenate([wsel_pat, thr_pat, biases]).astype(np.float32)


def _build_nc():
    import concourse.bass as bass
    import concourse.tile as tile
    from concourse import bacc, mybir

    nc = bacc.Bacc(
        "TRN2",
        target_bir_lowering=False,
        debug=False,
        enable_asserts=False,
    )
    f32 = mybir.dt.float32
    bf16 = mybir.dt.bfloat16
    f8 = mybir.dt.float8e4
    AF = mybir.ActivationFunctionType

    rq = nc.dram_tensor("rq", (D, 256), f8, kind="ExternalInput").ap()
    n_bias = 1 + len(_DIRECT_M)
    consts = nc.dram_tensor(
        "consts", (2 * S * NK + n_bias,), f32, kind="ExternalInput"
    ).ap()
    logits_out = nc.dram_tensor(
        "logits_out", (2, 128, S), bf16, kind="ExternalOutput"
    ).ap()

    with tile.TileContext(nc) as tc:
        with (
            tc.tile_pool(name="dram", bufs=1, space="DRAM") as dram,
            tc.tile_pool(name="rt", bufs=1) as rt_pool,
            tc.tile_pool(name="ri", bufs=1) as ri_pool,
            tc.tile_pool(name="cst", bufs=1) as cst_pool,
            tc.tile_pool(name="psum", bufs=2, space="PSUM") as psum_pool,
            tc.tile_pool(name="work", bufs=2) as work_pool,
            tc.tile_pool(name="pacc", bufs=2) as pacc_pool,
            tc.tile_pool(name="outs", bufs=2) as out_pool,
        ):
            # --- on-device AllGather of the 4 query shards -> full key matrix ---
            groups = [[0, 1, 2, 3], [4, 5, 6, 7]]
            # (A dummy warmup collective was tried to absorb the ~28us
            # first-collective firmware entry seen in traces - it made things
            # WORSE: the entry cost is per-launch and serializes.)
            halves = []  # (outb, d_row_base) per collective
            if _SPLIT_AG:
                for h in range(2):
                    inb = dram.tile([D // 2, 256], f8, tag=f"inb{h}")
                    outb = dram.tile([4, D // 2, 256], f8, tag=f"outb{h}")
                    nc.gpsimd.dma_start(
                        inb[:], rq[h * (D // 2) : (h + 1) * (D // 2), :]
                    )
                    nc.gpsimd.collective_compute(
                        "AllGather",
                        mybir.AluOpType.bypass,
                        replica_groups=groups,
                        ins=[inb.opt()],
                        outs=[outb.opt()],
                    )
                    halves.append((outb, h * (D // 2)))
            else:
                inb = dram.tile([D, 256], f8)
                outb = dram.tile([4, D, 256], f8)
                # NOTE: must stay on gpsimd — a sync-ring DMA here measured
                # ~12us SLOWER (the collective firmware tracks the gpsimd
                # engine-driven comm_in write; a ring DMA adds polling delay).
                nc.gpsimd.dma_start(inb[:], rq)
                nc.gpsimd.collective_compute(
                    "AllGather",
                    mybir.AluOpType.bypass,
                    replica_groups=groups,
                    ins=[inb.opt()],
                    outs=[outb.opt()],
                )
                halves.append((outb, 0))

            ri = []
            rt = []
            for dc in range(6):
                t2 = ri_pool.tile([128, 256], f8, tag=f"ri{dc}")
                nc.sync.dma_start(out=t2, in_=rq[dc * 128 : (dc + 1) * 128, :])
                ri.append(t2)
                t_ = rt_pool.tile([128, S * T], f8, tag=f"rt{dc}")
                row = dc * 128
                outb_h, base = halves[0] if row < D // 2 or not _SPLIT_AG else halves[1]
                if _SPLIT_AG and row >= D // 2:
                    outb_h, base = halves[1]
                # one 3D DMA per dc chunk (source rank-major, dest (p, r, x))
                nc.sync.dma_start(
                    out=t_.rearrange("p (r x) -> p r x", r=4),
                    in_=outb_h[:, row - base : row - base + 128, :].rearrange(
                        "r p x -> p r x"
                    ),
                )
                rt.append(t_)

            wsel_b = cst_pool.tile([128, S * NK], f32)
            nc.sync.dma_start(
                out=wsel_b,
                in_=bass.AP(
                    tensor=consts.tensor,
                    offset=consts.offset,
                    ap=[[0, 128], [1, S * NK]],
                ),
            )
            thr_b = cst_pool.tile([128, S * NK], f32)
            nc.sync.dma_start(
                out=thr_b,
                in_=bass.AP(
                    tensor=consts.tensor,
                    offset=consts.offset + S * NK,
                    ap=[[0, 128], [1, S * NK]],
                ),
            )
            negmu_b = cst_pool.tile([128, n_bias], f32)
            nc.sync.dma_start(
                out=negmu_b,
                in_=bass.AP(
                    tensor=consts.tensor,
                    offset=consts.offset + 2 * S * NK,
                    ap=[[0, 128], [1, n_bias]],
                ),
            )

            vec = nc.vector
            gps = nc.gpsimd if _USE_GPSIMD else nc.vector

            # Warm the activation table sets during the ramp with dummy 1-elem
            # Ln and Exp passes: walrus assigns sets per-activation, so
            # whichever loads it decides to emit land here (hidden behind the
            # collective wait) instead of at elementwise-phase start.
            actwu = cst_pool.tile([128, 1], f32)
            nc.scalar.activation(out=actwu, in_=thr_b[:, 0:1], func=AF.Ln)
            nc.scalar.activation(out=actwu, in_=actwu, func=AF.Exp)

            # --- sim matmuls into one 2-bank PSUM tile (128, 1024) per ip.
            # dc0-2 chains (AG half 1) issued for ALL (ip,nch) regions first,
            # so PE fills the wait for AG half 2 instead of stalling per
            # chain; the accumulation groups stay open until their dc5.
            pss = []
            for ip in range(2):
                ps_t = psum_pool.tile([128, 1024], f32, tag=f"sim{ip}")
                pss.append(ps_t)
            for dcg in (range(0, 3), range(3, 6)):
                for ip in range(2):
                    for nch in range(2):
                        for dc in dcg:
                            nc.tensor.matmul(
                                pss[ip][:, nch * 512 : (nch + 1) * 512],
                                lhsT=ri[dc][:, ip * 128 : (ip + 1) * 128],
                                rhs=rt[dc][:, nch * 512 : (nch + 1) * 512],
                                start=(dc == 0),
                                stop=(dc == 5),
                            )

            poolks = []
            for ip in range(2):
                ps = pss[ip]
                # --- ScalarE: Square + c0 first so DVE's reduce(c0) starts
                # while w/w2 are still computing; the first DVE mul needs the
                # 3rd/4th pass either way.  ScalarE idles during the DVE
                # phase, so chain powers m=7,9 (leaves) are also computed
                # here directly (Square with their own -mu bias + Exp),
                # dropping two DVE muls per tile.
                d = work_pool.tile([128, 1024], f32, tag="d")
                nc.scalar.activation(out=d, in_=ps, func=AF.Square, bias=negmu_b[:, 0:1], scale=1.0)
                c0 = work_pool.tile([128, 1024], bf16, tag="c0")
                nc.scalar.activation(out=c0, in_=d, func=AF.Exp, scale=-50.0)
                w = work_pool.tile([128, 1024], bf16, tag="w")
                nc.scalar.activation(out=w, in_=ps, func=AF.Exp, scale=-20.0)
                w2 = work_pool.tile([128, 1024], bf16, tag="w2")
                nc.scalar.activation(out=w2, in_=ps, func=AF.Exp, scale=-40.0)
                direct = {}
                for di, m in enumerate(_DIRECT_M):
                    dm = work_pool.tile([128, 1024], f32, tag=f"dm{m}")
                    nc.scalar.activation(
                        out=dm, in_=ps, func=AF.Square,
                        bias=negmu_b[:, di + 1 : di + 2], scale=1.0,
                    )
                    tm = work_pool.tile([128, 1024], bf16, tag=f"tm{m}")
                    nc.scalar.activation(out=tm, in_=dm, func=AF.Exp, scale=-50.0)
                    direct[m] = tm

                # --- geometric chain via even anchors: c_{2i} = c_{2i-2}*w2,
                # odd powers c_{2i+1} = c_{2i}*w -- 9 DVE muls, interleaved
                # with the 10 segmented reduces in dependency order so the
                # DVE queue never stalls on ScalarE.  All on DVE: GpSimd muls
                # are slow AND slow concurrent DVE ops ~4x (measured).  (A
                # fused tensor_tensor_reduce for the odd pools was tried: its
                # accum_out is hard-asserted scalar-only, no segmentation.)
                poolk = pacc_pool.tile([128, S, NK], f32)

                def _red(m, cm):
                    vec.reduce_sum(
                        out=poolk[:, :, m : m + 1],
                        in_=cm.rearrange("p (j q) -> p j q", q=T),
                        axis=mybir.AxisListType.X,
                    )

                _red(0, c0)
                ce = c0
                for i in range(5):
                    m_odd = 2 * i + 1
                    if m_odd in direct:
                        _red(m_odd, direct[m_odd])
                    else:
                        co = work_pool.tile([128, 1024], bf16, tag=f"c{m_odd}")
                        vec.tensor_mul(out=co, in0=ce, in1=w)
                        _red(m_odd, co)
                    if i < 4:
                        cn = work_pool.tile([128, 1024], bf16, tag=f"c{2*i+2}")
                        vec.tensor_mul(out=cn, in0=ce, in1=w2)
                        _red(2 * i + 2, cn)
                        ce = cn
                poolks.append(poolk)

            # --- Ke = ln(max(pool, thr)), logits = sum_k Ke*w_sel.  Both
            # ips' Ln tails AFTER all exps: walrus swaps activation table
            # sets at every Exp<->Ln transition (~1.3us each); grouping the
            # Lns leaves one in-phase swap instead of three.
            for ip in range(2):
                poolk = poolks[ip]
                pkf = poolk.rearrange("p j k -> p (j k)")
                vec.tensor_max(out=pkf, in0=pkf, in1=thr_b)
                ke = work_pool.tile([128, S * NK], f32, tag="ke")
                nc.scalar.activation(out=ke, in_=pkf, func=AF.Ln)
                vec.tensor_mul(out=ke, in0=ke, in1=wsel_b)
                lg = out_pool.tile([128, S], f32, tag="lg")
                vec.reduce_sum(
                    out=lg,
                    in_=ke.rearrange("p (j k) -> p j k", k=NK),
                    axis=mybir.AxisListType.X,
                )
                lgh = out_pool.tile([128, S], bf16, tag="lgh")
                vec.tensor_copy(out=lgh, in_=lg)
                nc.sync.dma_start(out=logits_out[ip], in_=lgh)
    nc.finalize()
    return nc


def _build_runner(nc, n_cores):
    """Mirror bass2jax.run_bass_via_pjrt's multi-core path, but build the
    shard_map jit ONCE and return a reusable callable (the library re-jits
    per call, costing ~0.45s of re-lowering each time)."""
    import jax
    from jax.sharding import Mesh, PartitionSpec

    import warnings

    with warnings.catch_warnings():
        warnings.simplefilter("ignore", DeprecationWarning)
        from jax.experimental.shard_map import shard_map
    from concourse import mybir
    from concourse.bass2jax import (
        _bass_exec_p,
        install_neuronx_cc_hook,
        partition_id_tensor,
    )

    install_neuronx_cc_hook()

    partition_name = nc.partition_id_tensor.name if nc.partition_id_tensor else None
    in_names, out_names, out_avals, zero_outs = [], [], [], []
    for alloc in nc.m.functions[0].allocations:
        if not isinstance(alloc, mybir.MemoryLocationSet):
            continue
        name = alloc.memorylocations[0].name
        if alloc.kind == "ExternalInput":
            if name != partition_name:
                in_names.append(name)
        elif alloc.kind == "ExternalOutput":
            out_names.append(name)
            shape = tuple(alloc.tensor_shape)
            dtype = mybir.dt.np(alloc.dtype)
            out_avals.append(jax.core.ShapedArray(shape, dtype))
            zero_outs.append(np.zeros(shape, dtype))
    n_params = len(in_names)
    n_outs = len(out_avals)
    in_names_full = list(in_names) + list(out_names)
    if partition_name is not None:
        in_names_full.append(partition_name)

    donate = tuple(range(n_params, n_params + n_outs))

    def _body(*args):
        operands = list(args)
        if partition_name is not None:
            operands.append(partition_id_tensor())
        outs = _bass_exec_p.bind(
            *operands,
            out_avals=tuple(out_avals),
            in_names=tuple(in_names_full),
            out_names=tuple(out_names),
            lowering_input_output_aliases=(),
            sim_require_finite=True,
            sim_require_nnan=True,
            nc=nc,
        )
        return tuple(outs)

    devices = jax.devices()[:n_cores]
    mesh = Mesh(np.asarray(devices), ("core",))
    in_specs = (PartitionSpec("core"),) * (n_params + n_outs)
    out_specs = (PartitionSpec("core"),) * len(out_names)
    sharded = jax.jit(
        shard_map(
            _body, mesh=mesh, in_specs=in_specs, out_specs=out_specs, check_rep=False
        ),
        donate_argnums=donate,
        keep_unused=True,
    )

    def run(in_maps, overlap_fn=None):
        per_core = [[np.asarray(m[name]) for name in in_names] for m in in_maps]
        concat_in = [
            np.concatenate([per_core[c][i] for c in range(n_cores)], axis=0)
            for i in range(n_params)
        ]
        concat_zeros = [
            np.zeros((n_cores * z.shape[0], *z.shape[1:]), z.dtype) for z in zero_outs
        ]
        out_arrs = sharded(*concat_in, *concat_zeros)
        # dispatch is async; host work passed via overlap_fn runs while the
        # device round-trip is in flight, before the blocking fetch below.
        overlap_result = overlap_fn() if overlap_fn is not None else None
        res = [
            {
                name: np.asarray(out_arrs[i]).reshape(n_cores, *out_avals[i].shape)[c]
                for i, name in enumerate(out_names)
            }
            for c in range(n_cores)
        ]
        return res, overlap_result

    return run


def _ensure_ready():
    if "run" in _STATE:
        return
    nc = _build_nc()
    run = _build_runner(nc, N_CORES)
    _STATE["nc"] = nc
    _STATE["run"] = run


def _warmup():
    """Full-pipeline warmup at import: traces + walrus-compiles the NEFF,
    loads it on all 8 cores, sets up the comm world, and warms the host-side
    numpy/BLAS paths, so the first real kernel() call is steady-state."""
    rng = np.random.RandomState(0)
    fake = {
        "claim_reps": rng.randn(B, T, D).astype(np.float32),
        "sentence_token_reps": rng.randn(B, S, T, D).astype(np.float32),
        "claim_token_mask": np.ones((B, T), dtype=bool),
        "token_mask": np.ones((B, S, T), dtype=bool),
        "w_sel": rng.randn(KERNEL, 1).astype(np.float32) * 0.02,
        "b_sel": np.zeros(1, np.float32),
        "w_g1": rng.randn(2 * D, 128).astype(np.float32) * 0.02,
        "b_g1": np.zeros(128, np.float32),
        "w_g2": rng.randn(128, 1).astype(np.float32) * 0.02,
        "b_g2": np.zeros(1, np.float32),
        "w_rat": rng.randn(KERNEL, 1).astype(np.float32) * 0.02,
        "b_rat": np.zeros(1, np.float32),
        "w_lab": rng.randn(2 * D, 3).astype(np.float32) * 0.02,
        "b_lab": np.zeros(3, np.float32),
    }
    kernel(**fake)


def _softmax(x, axis):
    m = np.max(x, axis=axis, keepdims=True)
    e = np.exp(x - m)
    return e / e.sum(axis=axis, keepdims=True)


def _node_part(reps, norms, claim_reps, w_rat, b_rat, w_g1, b_g1, w_lab):
    """Everything that does not depend on the device's logits: the node
    kernel -> rationale (B,S,1) plus the z-side matmul terms of the gating
    MLP and label head.  Runs while the device round-trip is in flight."""
    t_ = reps.shape[2]
    ncl = np.sqrt(np.einsum("btd,btd->bt", claim_reps, claim_reps))
    dotn = np.einsum("btd,bstd->bst", claim_reps, reps, optimize=True)
    simn = dotn / np.maximum(ncl[:, None, :] * norms, EPS)
    mu32 = MU.astype(np.float32)
    isig32 = (1.0 / SIGMA).astype(np.float32)
    rbfn = np.exp(-0.5 * ((simn[..., None] - mu32) * isig32) ** 2)
    pooln = rbfn * np.float32(t_)
    phi = np.mean(np.log(np.clip(pooln, CLAMP_MIN, None)), axis=-2)
    rationale = _softmax(phi @ w_rat + b_rat, axis=1)
    z = reps[:, :, 0, :]
    zw1 = z @ w_g1[:D] + b_g1  # (B,S2,128), broadcast over S1 in the tail
    zlab = z @ w_lab[D:]       # (B,S2,3)
    return rationale, z, zw1, zlab


def _edge_tail(reps, logits, overlap, w_g1, w_g2, b_g2, w_lab, b_lab):
    """Logits (B,S1,S2,T1) + precomputed logits-independent terms ->
    output (B,3). float32 numpy.  Assumes all-ones masks (the masked path
    goes through _reference_numpy).  cat([z_exp, z_hat]) @ W is split into
    z@W_top (precomputed in the overlap window, broadcast over i) +
    z_hat@W_bot -- halves the MLP flops and avoids the (B,S,S,2D) concat."""
    rationale, z, zw1, zlab = overlap
    attn = _softmax(logits, axis=3)
    z_hat = np.einsum("bjtd,bijt->bijd", reps, attn, optimize=True)
    h = np.maximum(z_hat @ w_g1[D:] + zw1[:, None, :, :], 0.0)
    beta = _softmax(h @ w_g2 + b_g2, axis=1)
    zb = np.sum(beta * z_hat, axis=1)
    slp = _softmax(zb @ w_lab[:D] + zlab + b_lab, axis=-1)
    return np.sum(slp * rationale, axis=1)


def _reference_numpy(claim_reps, sentence_token_reps, claim_token_mask, token_mask,
                     w_sel, b_sel, w_g1, b_g1, w_g2, b_g2, w_rat, b_rat,
                     w_lab, b_lab):
    """Pure-numpy fallback (used if masks are not all-ones or device fails)."""
    reps = sentence_token_reps.astype(np.float64)
    maskf = token_mask.astype(np.float64)
    b_, s_, t_, d_ = reps.shape
    norms = np.linalg.norm(reps, axis=-1)
    dot = np.einsum("bipd,bjqd->bijpq", reps, reps, optimize=True)
    sim = dot / np.maximum(norms[:, :, None, :, None] * norms[:, None, :, None, :], EPS)
    rbf = np.exp(-0.5 * ((sim[..., None] - MU) / SIGMA) ** 2)
    pool = rbf.sum(axis=4) * maskf[:, None, :, :, None]
    Ke = np.log(np.clip(pool, CLAMP_MIN, None))
    logits = Ke @ w_sel + b_sel
    m2 = np.broadcast_to(token_mask[:, None, :, :, None], logits.shape)
    lg = np.where(m2, logits, -10000.0)[..., 0]

    attn = _softmax(lg, axis=3)
    z_hat = np.einsum("bjtd,bijt->bijd", reps, attn, optimize=True)
    z = reps[:, :, 0, :]
    z_exp = np.broadcast_to(z[:, None, :, :], z_hat.shape)
    hcat = np.concatenate([z_exp, z_hat], axis=-1)
    h = np.maximum(hcat @ w_g1 + b_g1, 0.0)
    beta = _softmax(h @ w_g2 + b_g2, axis=1)
    v = np.concatenate([np.sum(beta * z_hat, axis=1), z], axis=-1)
    slp = _softmax(v @ w_lab + b_lab, axis=-1)

    ncl = np.linalg.norm(claim_reps, axis=-1)
    dotn = np.einsum("btd,bstd->bst", claim_reps, reps, optimize=True)
    simn = dotn / np.maximum(ncl[:, None, :] * norms, EPS)
    rbfn = np.exp(-0.5 * ((simn[..., None] - MU) / SIGMA) ** 2)
    pooln = rbfn * maskf[..., None] * float(t_)
    phi = np.mean(np.log(np.clip(pooln, CLAMP_MIN, None)), axis=-2)
    rationale = _softmax(phi @ w_rat + b_rat, axis=1)
    return (np.sum(slp * rationale, axis=1)).astype(np.float32)


def kernel(**inputs):
    global LAST_RESULTS
    LAST_RESULTS = None
    if any(not isinstance(v, np.ndarray) for v in inputs.values()):
        # jax-array inputs: one batched device_get pipelines the per-array
        # fetch latency instead of paying it 14x in the np.asarray calls.
        try:
            import jax

            inputs = jax.device_get(inputs)
        except Exception:
            pass
    claim_reps = np.asarray(inputs["claim_reps"], dtype=np.float32)
    reps = np.asarray(inputs["sentence_token_reps"], dtype=np.float32)
    claim_token_mask = np.asarray(inputs["claim_token_mask"])
    token_mask = np.asarray(inputs["token_mask"])
    w_sel = np.asarray(inputs["w_sel"], dtype=np.float32)
    b_sel = np.asarray(inputs["b_sel"], dtype=np.float32)
    w_g1 = np.asarray(inputs["w_g1"], dtype=np.float32)
    b_g1 = np.asarray(inputs["b_g1"], dtype=np.float32)
    w_g2 = np.asarray(inputs["w_g2"], dtype=np.float32)
    b_g2 = np.asarray(inputs["b_g2"], dtype=np.float32)
    w_rat = np.asarray(inputs["w_rat"], dtype=np.float32)
    b_rat = np.asarray(inputs["b_rat"], dtype=np.float32)
    w_lab = np.asarray(inputs["w_lab"], dtype=np.float32)
    b_lab = np.asarray(inputs["b_lab"], dtype=np.float32)

    if not (token_mask.all() and claim_token_mask.all()):
        return _reference_numpy(claim_reps, reps, claim_token_mask, token_mask,
                                w_sel, b_sel, w_g1, b_g1, w_g2, b_g2,
                                w_rat, b_rat, w_lab, b_lab)

    try:
        _ensure_ready()
        import ml_dtypes

        # --- host prep: normalize, bf16 D-major shards ---
        norms = np.sqrt(np.einsum("bstd,bstd->bst", reps, reps))
        rhat = reps / norms[..., None]
        rh16 = rhat.astype(ml_dtypes.float8_e4m3)  # (B,S,T,D)
        wk = _build_consts(w_sel)
        in_maps = []
        for c in range(N_CORES):
            b, ig = divmod(c, 4)
            shard = np.ascontiguousarray(
                rh16[b].reshape(S * T, D)[ig * 256 : (ig + 1) * 256, :].T
            )
            in_maps.append({"rq": shard, "consts": wk})

        _STATE["last_in_maps"] = in_maps
        res, overlap = _STATE["run"](
            in_maps,
            overlap_fn=lambda: _node_part(reps, norms, claim_reps, w_rat, b_rat,
                                          w_g1, b_g1, w_lab),
        )

        # --- gather: logits_out per core (2, 128, 16) -> (B, S1, S2, T1) ---
        # core c = b*4+ig; partition row = a*64 + t1; so the stacked
        # (8, 2, 128, 16) tensor factors as [b, ig, ip, a, t, j] with
        # i = ig*4 + ip*2 + a.
        L = np.stack([res[c]["logits_out"] for c in range(N_CORES)])
        logits = np.ascontiguousarray(
            L.reshape(B, 4, 2, 2, T, S).transpose(0, 1, 2, 3, 5, 4)
        ).reshape(B, S, S, T).astype(np.float32)
        # (dropped constants b_sel + sum_k w_k*C_k are uniform over T1 ->
        #  softmax-invariant)

        out = _edge_tail(reps, logits, overlap, w_g1, w_g2, b_g2, w_lab, b_lab)
        return out.astype(np.float32)
    except Exception as e:
        print(f"kernel device path failed ({e!r}); numpy fallback", file=sys.stderr)
        return _reference_numpy(claim_reps, reps, claim_token_mask, token_mask,
                                w_sel, b_sel, w_g1, b_g1, w_g2, b_g2,
                                w_rat, b_rat, w_lab, b_lab)


def profile_exec_time_ns():
    """Re-run the last device execution under the axon NTFF profiling hook
    and return max-over-cores exec_time_ns (neuron-profile's HW exec time).
    Returns None if profiling is unavailable.  Not used by kernel() itself."""
    import glob
    import tempfile

    in_maps = _STATE.get("last_in_maps")
    if in_maps is None or "run" not in _STATE:
        return None
    try:
        from trn_agent_boot.trn_boot import _ntff_profile_via_ctypes

        hook = _ntff_profile_via_ctypes("/opt/axon/libaxon_pjrt.so")
        if hook is None:
            return None
        neff_dir = tempfile.mkdtemp(prefix="ntff_")
        core_ids = list(range(N_CORES))
        with hook(neff_dir, core_ids):
            _STATE["run"](in_maps)
        if not glob.glob(os.path.join(neff_dir, "*.ntff")):
            return None
        import gauge.profiler
        from concourse._compat import FishPath

        profile = gauge.profiler.Profile(
            profile_path=FishPath(neff_dir),
            kernel_dev_mode=True,
            profile_on_exit=False,
            bass_kernel=_STATE["nc"].m,
            offline_processing=True,
            fname="*_body*",
        )
        results = profile.to_perfetto(model_index=tuple(core_ids))
        if not results:
            return None
        _STATE["last_traces"] = [r.trace_path for r in results]
        return max(r.exec_time_ns for r in results)
    except Exception as e:
        print(f"profile_exec_time_ns failed: {e!r}", file=sys.stderr)
        return None


try:
    _warmup()
except Exception as e:  # pragma: no cover - lazy retry inside kernel()
    print(f"kernel.py import-time warmup failed ({e!r}); will retry lazily",
          file=sys.stderr)

